# revision 1
# baseline (speedup 1.0000x reference)
"""Trainium2 Bass kernel for a dense transformer block (B=2, T=2048, C=1024,
H=16, HS=64, FF=4096, fp32, causal attention, scale=C**-0.5).

Sharding over 8 NeuronCores: core c -> batch g=c//4, rank r=c%4.
 - Attention: head-parallel (4 heads per core) in transposed-activation layout.
 - AllGather of per-head attention outputs within each 4-core batch group.
 - proj / LN2 / FFN: token-parallel (512 tokens per core); the rank-dependent
   token window of the gathered attention tensor is selected with a single
   partition_id-driven dynamic DMA so the program is identical on all cores.
All matmuls run as float32r (full fp32 data, 1 cycle/row at N>=256).
"""

import sys

import numpy as np

for _p in ("/opt/trn_rl_repo", "/root/.axon_site/_ro/trn_rl_repo"):
    if _p not in sys.path:
        sys.path.append(_p)

import concourse.bass as bass
import concourse.mybir as mybir
import concourse.tile as tile
from concourse import bacc
from concourse.bass_utils import run_bass_kernel_spmd

P = 128
B, T, C, H, HS, FF = 2, 2048, 1024, 16, 64, 4096
EPS = 1e-5
NCORES = 8
GSZ = 4            # cores per batch group
NHL = H // GSZ     # 4 local heads per core
TLOC = T // GSZ    # 512 tokens per core
KT = C // P        # 8 contraction tiles over C
NTT = T // P       # 16 token tiles
NJ = T // 512      # 4 q-chunks of 512
MTL = TLOC // P    # 4 local token tiles
MFF = FF // P      # 32 ff tiles
KF = FF // P       # 32 contraction tiles over FF

F32 = mybir.dt.float32
F32R = mybir.dt.float32r
REPLICA_GROUPS = [[0, 1, 2, 3], [4, 5, 6, 7]]


def _mm(nc, out, lhsT, rhs, start, stop):
    nc.tensor.matmul(
        out, lhsT.bitcast(F32R), rhs.bitcast(F32R), start=start, stop=stop
    )


def _ln_stats(nc, pool, x_t, eps_t):
    """mean/var over the free dim (C) of x_t [P, C] -> (rstd [P,1], nmr [P,1])
    with nmr = -mean*rstd."""
    xg = x_t.rearrange("p (s d) -> p s d", d=512)
    nsub = xg.shape[1]
    stats = pool.tile([P, nsub, 6], F32, tag="ln_stats", name="ln_stats")
    for s in range(nsub):
        nc.vector.bn_stats(out=stats[:, s, :], in_=xg[:, s, :])
    mv = pool.tile([P, 2], F32, tag="ln_mv", name="ln_mv")
    nc.vector.bn_aggr(out=mv[:], in_=stats[:])
    rstd = pool.tile([P, 1], F32, tag="ln_rstd", name="ln_rstd")
    nc.scalar.activation(
        out=rstd[:], in_=mv[:, 1:2], func=mybir.ActivationFunctionType.Sqrt,
        bias=eps_t[:],
    )
    nc.vector.reciprocal(out=rstd[:], in_=rstd[:])
    nmu = pool.tile([P, 1], F32, tag="ln_nmu", name="ln_nmu")
    nc.vector.tensor_scalar_mul(nmu[:], mv[:, 0:1], -1.0)
    return rstd, nmu


def _emit(tc, io, niter=1, fake_collective=False):
    for _it in range(niter):
        _emit_one(tc, io, fake_collective)


def _emit_one(tc, io, fake_collective=False):
    from contextlib import ExitStack

    nc = tc.nc
    ID = mybir.ActivationFunctionType.Identity
    EXP = mybir.ActivationFunctionType.Exp
    RELU = mybir.ActivationFunctionType.Relu

    with ExitStack() as top:
        consts = top.enter_context(tc.tile_pool(name="consts", bufs=1))
        dram = top.enter_context(tc.tile_pool(name="dram", bufs=1, space="DRAM"))
        stat_pool = top.enter_context(tc.tile_pool(name="stats", bufs=4))
        y_pool = top.enter_context(tc.tile_pool(name="ypool", bufs=1))

        eps_t = consts.tile([P, 1], F32)
        nc.vector.memset(eps_t[:], EPS)
        identr_t = consts.tile([P, P], F32R)
        nc.sync.dma_start(identr_t[:], io["ident"][:].bitcast(F32R))
        identr = identr_t[:]

        ag_in = dram.tile([NJ, NHL * HS, 512], F32)
        ag_out = dram.tile([NJ, C, 512], F32)

        y_sb = y_pool.tile([P, MTL, C], F32)
        y2_sb = y_pool.tile([P, MTL, C], F32)
        b1p_sb = y_pool.tile([P, MFF], F32)
        b2bc = y_pool.tile([P, C], F32)

        with ExitStack() as att_stack:
            attw = att_stack.enter_context(tc.tile_pool(name="attw", bufs=1))
            big = att_stack.enter_context(tc.tile_pool(name="attbig", bufs=1))

            qT_sb = big.tile([P, 2, T], F32R)
            kT_sb = big.tile([P, 2, T], F32R)
            v_sb = big.tile([P, NTT, NHL, HS + 1], F32R)

            # ----- Phases A+B: ln1 -> hT (SBUF-resident) -> QKV -----
            with ExitStack() as ph:
                hTp = ph.enter_context(tc.tile_pool(name="hTp", bufs=1))
                xpool = ph.enter_context(tc.tile_pool(name="xin", bufs=2))
                hpool = ph.enter_context(tc.tile_pool(name="happ", bufs=2))
                tpsum = ph.enter_context(
                    tc.tile_pool(name="tpsum", bufs=4, space="PSUM")
                )
                qkpsum = ph.enter_context(
                    tc.tile_pool(name="qkpsum", bufs=4, space="PSUM")
                )
                hT_sb = hTp.tile([P, KT, T], F32R)

                def ln1_tile(ti):
                    x_t = xpool.tile([P, C], F32, tag="x_t", name="x_t")
                    nc.sync.dma_start(x_t[:], io["x_full"][ti * P:(ti + 1) * P, :])
                    rstd, nmu = _ln_stats(nc, stat_pool, x_t, eps_t)
                    h_t = hpool.tile([P, C], F32R, tag="h_t", name="h_t")
                    nc.vector.tensor_scalar(
                        out=h_t[:], in0=x_t[:], scalar1=nmu[:], scalar2=rstd[:],
                        op0=mybir.AluOpType.add, op1=mybir.AluOpType.mult,
                    )
                    for kq in range(2):
                        ps = tpsum.tile([P, 4, P], F32R, tag="tp", name="tp")
                        for k2 in range(4):
                            kt = 4 * kq + k2
                            nc.tensor.transpose(
                                ps[:, k2, :], h_t[:, kt * P:(kt + 1) * P], identr,
                            )
                        nc.scalar.copy(
                            hT_sb[:, 4 * kq:4 * kq + 4, ti * P:(ti + 1) * P],
                            ps[:],
                        )

                # weights staged while ln1 streams
                wq_sb = attw.tile([P, KT, 2, P], F32R)
                nc.sync.dma_start(wq_sb[:], io["wq"][:].bitcast(F32R))
                wk_sb = attw.tile([P, KT, 2, P], F32R)
                nc.sync.dma_start(wk_sb[:], io["wk"][:].bitcast(F32R))
                wv_sb = attw.tile([P, KT, NHL * HS], F32R)
                nc.sync.dma_start(wv_sb[:], io["wv"][:].bitcast(F32R))
                bq_sb = attw.tile([P, 2], F32)
                nc.sync.dma_start(bq_sb[:], io["bq"][:])
                bk_sb = attw.tile([P, 2], F32)
                nc.sync.dma_start(bk_sb[:], io["bk"][:])
                bvbc = attw.tile([P, NHL * HS], F32)
                nc.sync.dma_start(
                    bvbc[:], io["bv"][None, :].to_broadcast((P, NHL * HS))
                )
                ones_c = attw.tile([P, NHL, 1], F32)
                nc.vector.memset(ones_c[:], 1.0)

                for ti in range(NTT):
                    ln1_tile(ti)
                for J in range(NJ):
                    for m in range(4):
                        mt = 4 * J + m
                        vps = qkpsum.tile([P, 512], F32, tag="qkv_ps", name="vps")
                        for kt in range(KT):
                            _mm(nc, vps[:, :NHL * HS],
                                hT_sb[:, kt, mt * P:(mt + 1) * P],
                                wv_sb[:, kt, :], kt == 0, kt == KT - 1)
                        nc.vector.tensor_add(
                            v_sb[:, mt, :, 0:HS],
                            vps[:, :NHL * HS].rearrange(
                                "p (h d) -> p h d", h=NHL
                            ),
                            bvbc[:].rearrange("p (h d) -> p h d", h=NHL),
                        )
                        nc.vector.tensor_copy(
                            v_sb[:, mt, :, HS:HS + 1], ones_c[:]
                        )
                    for pair in range(2):
                        qps = qkpsum.tile([P, 512], F32, tag="qkv_ps", name="qps")
                        for kt in range(KT):
                            _mm(nc, qps[:], wq_sb[:, kt, pair, :],
                                hT_sb[:, kt, J * 512:(J + 1) * 512],
                                kt == 0, kt == KT - 1)
                        nc.scalar.activation(
                            out=qT_sb[:, pair, J * 512:(J + 1) * 512], in_=qps[:],
                            func=ID, bias=bq_sb[:, pair:pair + 1],
                        )
                        kps = qkpsum.tile([P, 512], F32, tag="qkv_ps", name="kps")
                        for kt in range(KT):
                            _mm(nc, kps[:], wk_sb[:, kt, pair, :],
                                hT_sb[:, kt, J * 512:(J + 1) * 512],
                                kt == 0, kt == KT - 1)
                        nc.scalar.activation(
                            out=kT_sb[:, pair, J * 512:(J + 1) * 512], in_=kps[:],
                            func=ID, bias=bk_sb[:, pair:pair + 1],
                        )

            # wo/xpb prefetch into the space hT freed, overlapping attention
            wpre = att_stack.enter_context(tc.tile_pool(name="wpre", bufs=1))
            xpb_sb = wpre.tile([P, MTL, C], F32)
            nc.sync.dma_start(xpb_sb[:], io["xpb"][:])
            wo_sb = wpre.tile([P, KT, C], F32R)
            nc.sync.dma_start(wo_sb[:], io["wo"][:].bitcast(F32R))

            # causal masks for the 4 diagonal offsets: keep where q >= k,
            # i.e. (col + 512J) - (row + 128i) >= 0 with d = i - 4J.
            masks = attw.tile([P, 4, 512], F32)
            nc.sync.dma_start(masks[:], io["masks"][:])

            # ----- Phase C: causal attention, AllGather per q-chunk -----
            with ExitStack() as ph:
                stpsum = ph.enter_context(
                    tc.tile_pool(name="stpsum", bufs=3, space="PSUM")
                )
                upsum = ph.enter_context(
                    tc.tile_pool(name="upsum", bufs=2, space="PSUM")
                )
                ppool = ph.enter_context(tc.tile_pool(name="pT", bufs=4))
                mpool = ph.enter_context(tc.tile_pool(name="mtmp", bufs=2))
                npool = ph.enter_context(tc.tile_pool(name="norm", bufs=2))
                for J in range(NJ):
                    for h in range(NHL):
                        pair, off = h // 2, HS * (h % 2)
                        nk = 4 * J + 4
                        ups = upsum.tile([P, 512], F32, tag="ups", name="ups")
                        for ip in range(nk // 2):
                            i0 = 2 * ip
                            sps = stpsum.tile([P, 2, 512], F32, tag="sps",
                                              name="sps")
                            for j in range(2):
                                i = i0 + j
                                _mm(nc, sps[:, j, :],
                                    kT_sb[off:off + HS, pair, i * P:(i + 1) * P],
                                    qT_sb[off:off + HS, pair,
                                          J * 512:(J + 1) * 512],
                                    True, True)
                            pT = ppool.tile([P, 2, 512], F32R, tag="pT", name="pT")
                            if i0 >= 4 * J:
                                d = i0 - 4 * J
                                tmp = mpool.tile([P, 2, 512], F32, tag="mtmp",
                                                 name="mtmp")
                                nc.vector.tensor_add(
                                    tmp[:], sps[:], masks[:, d:d + 2, :]
                                )
                                nc.scalar.activation(out=pT[:], in_=tmp[:], func=EXP)
                            else:
                                nc.scalar.activation(out=pT[:], in_=sps[:], func=EXP)
                            for j in range(2):
                                i = i0 + j
                                _mm(nc, ups[:HS + 1, :], v_sb[:, i, h, :],
                                    pT[:, j, :], i == 0, i == nk - 1)
                        recip = npool.tile([1, 512], F32, tag="recip", name="recip")
                        nc.vector.reciprocal(recip[:], ups[HS:HS + 1, :])
                        rbc = npool.tile([HS, 512], F32, tag="rbc", name="rbc")
                        nc.gpsimd.partition_broadcast(rbc[:], recip[:])
                        att = npool.tile([HS, 512], F32, tag="att", name="att")
                        nc.vector.tensor_mul(att[:], ups[0:HS, :], rbc[:])
                        nc.sync.dma_start(
                            ag_in[J, h * HS:(h + 1) * HS, :], att[:]
                        )
                    if fake_collective:
                        for rr in range(GSZ):
                            nc.sync.dma_start(
                                ag_out[J, rr * NHL * HS:(rr + 1) * NHL * HS, :],
                                ag_in[J],
                            )
                    else:
                        nc.gpsimd.collective_compute(
                            "AllGather", mybir.AluOpType.bypass,
                            replica_groups=REPLICA_GROUPS,
                            ins=[ag_in[J].opt()], outs=[ag_out[J].opt()],
                        )

            # ----- Phase D: dynamic gather + proj + residual -----
            with ExitStack() as ph:
                agp = ph.enter_context(tc.tile_pool(name="agp", bufs=1))
                prpsum = ph.enter_context(
                    tc.tile_pool(name="prpsum", bufs=3, space="PSUM")
                )
                nc.sync.dma_start(b1p_sb[:], io["b1p"][:])
                nc.sync.dma_start(b2bc[:], io["b2"][None, :].to_broadcast((P, C)))
                pid = nc.sync.partition_id()
                rsel = pid % GSZ
                ag_sb = agp.tile([P, KT, TLOC], F32R)
                ag_view = ag_out[:].bitcast(F32R).rearrange(
                    "j (kt p) t -> p j kt t", p=P
                )
                for mt in range(MTL):
                    for kt in range(KT):
                        nc.sync.dma_start(
                            ag_sb[:, kt, mt * P:(mt + 1) * P],
                            ag_view[:, bass.ds(rsel, 1), kt,
                                    mt * P:(mt + 1) * P],
                        )
                for mt in range(MTL):
                    for nt in range(2):
                        pps = prpsum.tile([P, 512], F32, tag="pps", name="pps")
                        for kt in range(KT):
                            _mm(nc, pps[:], ag_sb[:, kt, mt * P:(mt + 1) * P],
                                wo_sb[:, kt, nt * 512:(nt + 1) * 512],
                                kt == 0, kt == KT - 1)
                        nc.vector.tensor_add(
                            y_sb[:, mt, nt * 512:(nt + 1) * 512], pps[:],
                            xpb_sb[:, mt, nt * 512:(nt + 1) * 512],
                        )
                        nc.vector.tensor_add(
                            y2_sb[:, mt, nt * 512:(nt + 1) * 512],
                            y_sb[:, mt, nt * 512:(nt + 1) * 512],
                            b2bc[:, nt * 512:(nt + 1) * 512],
                        )

        # ---------- Phases E..G ----------
        with ExitStack() as tail:
            tailp = tail.enter_context(tc.tile_pool(name="tailp", bufs=1))
            yT_sb = tailp.tile([P, KT, TLOC], F32R)
            rT = tailp.tile([P, MFF, TLOC], F32R)

            # ----- Phase E: ln2 + transpose -----
            with ExitStack() as ph:
                yhp = ph.enter_context(tc.tile_pool(name="yh", bufs=2))
                tp2 = ph.enter_context(tc.tile_pool(name="tp2", bufs=4, space="PSUM"))
                for mt in range(MTL):
                    rstd, nmu = _ln_stats(nc, stat_pool, y_sb[:, mt, :], eps_t)
                    yh = yhp.tile([P, C], F32R, tag="yh", name="yh")
                    nc.vector.tensor_scalar(
                        out=yh[:], in0=y_sb[:, mt, :], scalar1=nmu[:],
                        scalar2=rstd[:],
                        op0=mybir.AluOpType.add, op1=mybir.AluOpType.mult,
                    )
                    for kq in range(2):
                        ps = tp2.tile([P, 4, P], F32R, tag="tp2", name="tp2")
                        for k2 in range(4):
                            kt = 4 * kq + k2
                            nc.tensor.transpose(
                                ps[:, k2, :], yh[:, kt * P:(kt + 1) * P], identr
                            )
                        nc.scalar.copy(
                            yT_sb[:, 4 * kq:4 * kq + 4, mt * P:(mt + 1) * P],
                            ps[:],
                        )

            # ----- Phase F: FFN1 (relu(yT @ W1 + b1) -> rT) -----
            w2p = tail.enter_context(tc.tile_pool(name="w2p", bufs=3))
            with ExitStack() as ph:
                w1p = ph.enter_context(tc.tile_pool(name="w1p", bufs=3))
                zps_p = ph.enter_context(
                    tc.tile_pool(name="zps", bufs=4, space="PSUM")
                )
                w1_view = io["w1"].bitcast(F32R).rearrange(
                    "(kt p) f -> p kt f", p=P
                )
                for mf in range(MFF):
                    w1_t = w1p.tile([P, KT, P], F32R, tag="w1t", name="w1t")
                    nc.sync.dma_start(w1_t[:], w1_view[:, :, mf * P:(mf + 1) * P])
                    for half in range(2):
                        hs_sl = slice(half * 256, (half + 1) * 256)
                        zps = zps_p.tile([P, 256], F32, tag="zps", name="zps")
                        for kt in range(KT):
                            _mm(nc, zps[:], w1_t[:, kt, :],
                                yT_sb[:, kt, hs_sl],
                                kt == 0, kt == KT - 1)
                        nc.scalar.activation(
                            out=rT[:, mf, hs_sl], in_=zps[:], func=RELU,
                            bias=b1p_sb[:, mf:mf + 1],
                        )

            # ----- Phase G: FFN2 + residual + out -----
            with ExitStack() as ph:
                fps_p = ph.enter_context(
                    tc.tile_pool(name="fps", bufs=1, space="PSUM")
                )
                otmp = ph.enter_context(tc.tile_pool(name="otmp", bufs=3))
                fps = [
                    [
                        fps_p.tile(
                            [P, 512], F32, tag=f"fps_{mt}_{nt}",
                            name=f"fps_{mt}_{nt}",
                        )
                        for nt in range(2)
                    ]
                    for mt in range(MTL)
                ]
                for kt in range(KF):
                    w2_t = w2p.tile([P, C], F32R, tag="w2t", name="w2t")
                    nc.sync.dma_start(
                        w2_t[:], io["w2"][kt * P:(kt + 1) * P, :].bitcast(F32R)
                    )
                    for mt in range(MTL):
                        for nt in range(2):
                            _mm(nc, fps[mt][nt][:],
                                rT[:, kt, mt * P:(mt + 1) * P],
                                w2_t[:, nt * 512:(nt + 1) * 512],
                                kt == 0, kt == KF - 1)
                for mt in range(MTL):
                    for nt in range(2):
                        t1 = otmp.tile([P, 512], F32, tag="otmp", name="otmp")
                        nc.vector.tensor_add(
                            t1[:], fps[mt][nt][:],
                            y2_sb[:, mt, nt * 512:(nt + 1) * 512],
                        )
                        nc.sync.dma_start(
                            io["out"][mt * P:(mt + 1) * P, nt * 512:(nt + 1) * 512],
                            t1[:],
                        )



def build_nc(niter=1, fake_collective=False):
    nc = bacc.Bacc(None, target_bir_lowering=False, debug=False, num_devices=NCORES)
    io = {}
    io["x_full"] = nc.dram_tensor("x_full", [T, C], F32, kind="ExternalInput").ap()
    io["xpb"] = nc.dram_tensor("xpb", [P, MTL, C], F32, kind="ExternalInput").ap()
    io["wq"] = nc.dram_tensor("wq", [P, KT, 2, P], F32, kind="ExternalInput").ap()
    io["wk"] = nc.dram_tensor("wk", [P, KT, 2, P], F32, kind="ExternalInput").ap()
    io["wv"] = nc.dram_tensor(
        "wv", [P, KT, NHL * HS], F32, kind="ExternalInput"
    ).ap()
    io["bq"] = nc.dram_tensor("bq", [P, 2], F32, kind="ExternalInput").ap()
    io["bk"] = nc.dram_tensor("bk", [P, 2], F32, kind="ExternalInput").ap()
    io["bv"] = nc.dram_tensor("bv", [NHL * HS], F32, kind="ExternalInput").ap()
    io["wo"] = nc.dram_tensor("wo", [P, KT, C], F32, kind="ExternalInput").ap()
    io["w1"] = nc.dram_tensor("w1", [C, FF], F32, kind="ExternalInput").ap()
    io["b1p"] = nc.dram_tensor("b1p", [P, MFF], F32, kind="ExternalInput").ap()
    io["w2"] = nc.dram_tensor("w2", [FF, C], F32, kind="ExternalInput").ap()
    io["b2"] = nc.dram_tensor("b2", [C], F32, kind="ExternalInput").ap()
    io["ident"] = nc.dram_tensor("ident", [P, P], F32, kind="ExternalInput").ap()
    io["masks"] = nc.dram_tensor(
        "masks", [P, 4, 512], F32, kind="ExternalInput"
    ).ap()
    io["out"] = nc.dram_tensor("out", [TLOC, C], F32, kind="ExternalOutput").ap()
    with tile.TileContext(nc) as tc:
        _emit(tc, io, niter, fake_collective)
    nc.compile()
    return nc


def host_prep(inputs):
    """Fold layernorm affines / biases / attention scale into the weights and
    build the 8 per-core input maps."""
    f = np.float32
    x = np.ascontiguousarray(inputs["x"], f)
    Wq, Wk, Wv = (np.asarray(inputs[k], f) for k in ("Wq", "Wk", "Wv"))
    Wo, bo = np.asarray(inputs["Wo"], f), np.asarray(inputs["bo"], f)
    W1, b1 = np.asarray(inputs["W1"], f), np.asarray(inputs["b1"], f)
    W2, b2 = np.asarray(inputs["W2"], f), np.asarray(inputs["b2"], f)
    g1, be1 = np.asarray(inputs["g1"], f), np.asarray(inputs["be1"], f)
    g2, be2 = np.asarray(inputs["g2"], f), np.asarray(inputs["be2"], f)

    scale = f(C) ** f(-0.5)
    Wq_f = (g1[None, :, None] * Wq) * scale
    Wk_f = g1[None, :, None] * Wk
    Wv_f = g1[None, :, None] * Wv
    bq = np.einsum("c,hcd->hd", be1, Wq).astype(f) * scale
    bk = np.einsum("c,hcd->hd", be1, Wk).astype(f)
    bv = np.einsum("c,hcd->hd", be1, Wv).astype(f)
    W1_f = np.ascontiguousarray(g2[:, None] * W1, f)
    b1p = (b1 + be2 @ W1).astype(f)
    Wo_c = np.ascontiguousarray(Wo.reshape(KT, P, C).transpose(1, 0, 2), f)
    W2_c = np.ascontiguousarray(W2, f)
    b1p_dev = np.ascontiguousarray(b1p.reshape(MFF, P).T)

    ident_np = np.eye(P, dtype=f)
    rr = np.arange(P)[:, None]
    cc = np.arange(512)[None, :]
    masks_np = np.stack(
        [np.where(cc - rr - 128 * d >= 0, 0.0, -1e9).astype(f) for d in range(4)],
        axis=1,
    )

    in_maps = []
    for c in range(NCORES):
        g, r = divmod(c, GSZ)
        hs = [GSZ * r + j for j in range(NHL)]
        wq_pairs = np.stack(
            [np.concatenate([Wq_f[hs[2 * p]], Wq_f[hs[2 * p + 1]]], axis=1)
             for p in range(2)]
        )
        wk_pairs = np.stack(
            [np.concatenate([Wk_f[hs[2 * p]], Wk_f[hs[2 * p + 1]]], axis=1)
             for p in range(2)]
        )
        bq_pairs = np.stack(
            [np.concatenate([bq[hs[2 * p]], bq[hs[2 * p + 1]]]) for p in range(2)]
        )
        bk_pairs = np.stack(
            [np.concatenate([bk[hs[2 * p]], bk[hs[2 * p + 1]]]) for p in range(2)]
        )
        wv_cat = np.concatenate([Wv_f[h] for h in hs], axis=1)
        xpb = x[g, TLOC * r:TLOC * (r + 1)] + bo
        in_maps.append({
            "x_full": np.ascontiguousarray(x[g]),
            "xpb": np.ascontiguousarray(
                xpb.reshape(MTL, P, C).transpose(1, 0, 2)
            ),
            "wq": np.ascontiguousarray(
                wq_pairs.reshape(2, KT, P, P).transpose(2, 1, 0, 3)
            ),
            "wk": np.ascontiguousarray(
                wk_pairs.reshape(2, KT, P, P).transpose(2, 1, 0, 3)
            ),
            "wv": np.ascontiguousarray(
                wv_cat.reshape(KT, P, NHL * HS).transpose(1, 0, 2)
            ),
            "bq": np.ascontiguousarray(bq_pairs.T),
            "bk": np.ascontiguousarray(bk_pairs.T),
            "bv": np.ascontiguousarray(np.concatenate([bv[h] for h in hs])),
            "wo": Wo_c,
            "w1": W1_f,
            "b1p": b1p_dev,
            "w2": W2_c,
            "b2": b2,
            "ident": ident_np,
            "masks": masks_np,
        })
    return in_maps


_NC = None


def _get_nc():
    global _NC
    if _NC is None:
        _NC = build_nc()
    return _NC


def kernel(**inputs) -> np.ndarray:
    nc = _get_nc()
    in_maps = host_prep(inputs)
    res = run_bass_kernel_spmd(nc, in_maps, core_ids=list(range(NCORES)))
    out = np.empty((B, T, C), np.float32)
    for c in range(NCORES):
        g, r = divmod(c, GSZ)
        out[g, TLOC * r:TLOC * (r + 1)] = res.results[c]["out"]
    return out



# revision 27
# speedup vs baseline: 2.3711x; 2.3711x over previous
"""Trainium2 Bass kernel for a dense transformer block (B=2, T=2048, C=1024,
H=16, HS=64, FF=4096, fp32 io, causal attention, scale=C**-0.5).

Sharding over 8 NeuronCores: core c -> batch g=c//4, rank r=c%4.
 - Attention: head-parallel (4 heads per core, as 2 packed pairs).
 - AllGather (bf16) of per-head attention outputs within each 4-core group.
 - proj / LN2 / FFN: token-parallel (512 tokens per core) via one dynamic
   partition_id-driven gather DMA per contraction tile.

v2 vs baseline:
 - all matmul operands bf16 (PSUM accum + softmax + LN stats stay fp32)
 - transposes moved off PE onto the DMA xbar (dma_start_transpose)
 - score matmuls 2-head packed into the PE array via tile_position (K=64)
 - causal diagonal chunks width-split (skip fully-masked blocks)
 - ACT runs one activation table per phase (ID -> Exp -> Relu)
 - weights pre-tiled host-side for contiguous >=1KB DMA lines; W1 fully
   prefetched during attention, W2 streamed during FFN1/2.
"""

import sys

import numpy as np

for _p in ("/opt/trn_rl_repo", "/root/.axon_site/_ro/trn_rl_repo"):
    if _p not in sys.path:
        sys.path.append(_p)

import concourse.bass as bass
import concourse.mybir as mybir
import concourse.tile as tile
from concourse import bacc
from concourse.bass_utils import run_bass_kernel_spmd

import ml_dtypes

BF16NP = ml_dtypes.bfloat16

P = 128
B, T, C, H, HS, FF = 2, 2048, 1024, 16, 64, 4096
EPS = 1e-5
NCORES = 8
GSZ = 4            # cores per batch group
NHL = H // GSZ     # 4 local heads per core
NPAIR = NHL // 2   # 2 head pairs per core
TLOC = T // GSZ    # 512 tokens per core
KT = C // P        # 8 contraction tiles over C
NTT = T // P       # 16 token tiles
NJ = T // 512      # 4 q-chunks of 512
MTL = TLOC // P    # 4 local token tiles
MFF = FF // P      # 32 ff tiles
KF = FF // P       # 32 contraction tiles over FF

F32 = mybir.dt.float32
BF16 = mybir.dt.bfloat16
REPLICA_GROUPS = [[0, 1, 2, 3], [4, 5, 6, 7]]

ID = mybir.ActivationFunctionType.Identity
EXP = mybir.ActivationFunctionType.Exp
RELU = mybir.ActivationFunctionType.Relu


def _ln_stats(nc, pool, x_t, eps_t):
    """mean/var over the free dim (C) of x_t [P, C] -> (rstd [P,1], nmr [P,1])
    with nmu = -mean."""
    xg = x_t.rearrange("p (s d) -> p s d", d=512)
    nsub = xg.shape[1]
    stats = pool.tile([P, nsub, 6], F32, tag="ln_stats", name="ln_stats")
    for s in range(nsub):
        nc.vector.bn_stats(out=stats[:, s, :], in_=xg[:, s, :])
    mv = pool.tile([P, 2], F32, tag="ln_mv", name="ln_mv")
    nc.vector.bn_aggr(out=mv[:], in_=stats[:])
    rstd = pool.tile([P, 1], F32, tag="ln_rstd", name="ln_rstd")
    nc.scalar.activation(
        out=rstd[:], in_=mv[:, 1:2], func=mybir.ActivationFunctionType.Sqrt,
        bias=eps_t[:],
    )
    nc.vector.reciprocal(out=rstd[:], in_=rstd[:])
    nmu = pool.tile([P, 1], F32, tag="ln_nmu", name="ln_nmu")
    nc.vector.tensor_scalar_mul(nmu[:], mv[:, 0:1], -1.0)
    return rstd, nmu



def _ln_stats_gp(nc, pool, x_t, eps_t, scratch):
    """LN stats on GPSIMD via accum_out sums; sqrt on ACT, recip on DVE
    (tiny [P,1] ops). Returns (rstd, nmu)."""
    C_ = x_t.free_size()
    s1 = pool.tile([P, 1], F32, tag="gp_s1", name="gp_s1")
    s2 = pool.tile([P, 1], F32, tag="gp_s2", name="gp_s2")
    nc.gpsimd.scalar_tensor_tensor(
        out=scratch[:], in0=x_t, scalar=0.0, in1=x_t,
        op0=mybir.AluOpType.mult, op1=mybir.AluOpType.add, accum_out=s1[:],
    )
    nc.gpsimd.scalar_tensor_tensor(
        out=scratch[:], in0=x_t, scalar=1.0, in1=x_t,
        op0=mybir.AluOpType.mult, op1=mybir.AluOpType.mult, accum_out=s2[:],
    )
    nmu = pool.tile([P, 1], F32, tag="gp_nmu", name="gp_nmu")
    nc.gpsimd.tensor_scalar_mul(nmu[:], s1[:], -1.0 / C_)
    e2 = pool.tile([P, 1], F32, tag="gp_e2", name="gp_e2")
    nc.gpsimd.tensor_scalar_mul(e2[:], s2[:], 1.0 / C_)
    mu2 = pool.tile([P, 1], F32, tag="gp_mu2", name="gp_mu2")
    nc.gpsimd.tensor_tensor(
        out=mu2[:], in0=nmu[:], in1=nmu[:], op=mybir.AluOpType.mult
    )
    var = pool.tile([P, 1], F32, tag="gp_var", name="gp_var")
    nc.gpsimd.tensor_tensor(
        out=var[:], in0=e2[:], in1=mu2[:], op=mybir.AluOpType.subtract
    )
    rstd = pool.tile([P, 1], F32, tag="gp_rstd", name="gp_rstd")
    nc.scalar.activation(
        out=rstd[:], in_=var[:], func=mybir.ActivationFunctionType.Sqrt,
        bias=eps_t[:],
    )
    nc.vector.reciprocal(out=rstd[:], in_=rstd[:])
    return rstd, nmu


def _emit(tc, io, niter=1, fake_collective=False):
    for _it in range(niter):
        _emit_one(tc, io, fake_collective)


def _emit_one(tc, io, fake_collective=False):
    from contextlib import ExitStack

    nc = tc.nc

    with ExitStack() as top:
        consts = top.enter_context(tc.tile_pool(name="consts", bufs=1))
        dram = top.enter_context(tc.tile_pool(name="dram", bufs=1, space="DRAM"))
        stat_pool = top.enter_context(tc.tile_pool(name="stats", bufs=4))
        y_pool = top.enter_context(tc.tile_pool(name="ypool", bufs=1))
        w1pool = top.enter_context(tc.tile_pool(name="w1pool", bufs=1))

        eps_t = consts.tile([P, 1], F32)
        nc.vector.memset(eps_t[:], EPS)

        ag_in = dram.tile([NJ, NHL * HS, 512], BF16)
        ag_out = dram.tile([NJ, C, 512], BF16)

        y_sb = y_pool.tile([P, MTL, C], F32)
        b1p_sb = y_pool.tile([P, MFF], F32)
        b2bc_sb = y_pool.tile([P, C], F32)
        xpb_sb = y_pool.tile([P, MTL, C], BF16)



        with ExitStack() as att_stack:
            attw = att_stack.enter_context(tc.tile_pool(name="attw", bufs=1))
            big = att_stack.enter_context(tc.tile_pool(name="attbig", bufs=1))

            qT_sb = big.tile([P, NPAIR, T], BF16)
            kT_sb = big.tile([P, NPAIR, T], BF16)
            v_sb = big.tile([P, NTT, NHL, HS + 1], BF16)

            # weights staged behind the x stream (one DMA per tensor
            # to keep SP.SEQ free: each DMA costs ~565ns of sequencer time)
            wq_sb = attw.tile([P, KT, NPAIR, P], BF16)
            wk_sb = attw.tile([P, KT, NPAIR, P], BF16)
            wv_sb = attw.tile([P, KT, NHL * HS], BF16)
            bq_sb = attw.tile([P, NPAIR], F32)
            bk_sb = attw.tile([P, NPAIR], F32)
            bvbc = attw.tile([P, NHL * HS], F32)
            mask2 = attw.tile([P, 2, P], F32)
            wo_sb = y_pool.tile([P, KT, C], BF16)
            nc.vector.memset(v_sb[:, :, :, HS:HS + 1], 1.0)

            # ----- Phases A+B fused per q-chunk J: ln1 -> hT -> QKV(J) -----
            with ExitStack() as ph:
                hTp = ph.enter_context(tc.tile_pool(name="hTp", bufs=1))
                xpool = ph.enter_context(tc.tile_pool(name="xin", bufs=3))
                hpool = ph.enter_context(tc.tile_pool(name="happ", bufs=3))
                qkpsum = ph.enter_context(
                    tc.tile_pool(name="qkpsum", bufs=4, space="PSUM")
                )
                hT_sb = hTp.tile([P, KT, T], BF16)
                xv = io["x"].rearrange("(n p) c -> p n c", p=P)

                def ln1_pair(tp):
                    """load token tiles 2tp,2tp+1 in one DMA; ln1 + transpose
                    each."""
                    x2 = xpool.tile([P, 2, C], BF16, tag="x_t", name="x_t")
                    nc.sync.dma_start(x2[:], xv[:, 2 * tp:2 * tp + 2, :])
                    for s in range(2):
                        ti = 2 * tp + s
                        rstd, nmu = _ln_stats(nc, stat_pool, x2[:, s, :], eps_t)
                        h_t = hpool.tile([P, C], BF16, tag="h_t", name="h_t")
                        nc.vector.tensor_scalar(
                            out=h_t[:], in0=x2[:, s, :], scalar1=nmu[:],
                            scalar2=rstd[:],
                            op0=mybir.AluOpType.add, op1=mybir.AluOpType.mult,
                        )
                        nc.scalar.dma_start_transpose(
                            hT_sb[:, :, ti * P:(ti + 1) * P], h_t[:]
                        )

                ln1_pair(0)
                ln1_pair(1)
                # qkv weights now; they arrive behind the first x tiles
                nc.sync.dma_start(wv_sb[:], io["wv"][:])
                nc.sync.dma_start(wq_sb[:], io["wq"][:])
                nc.sync.dma_start(wk_sb[:], io["wk"][:])
                nc.sync.dma_start(bq_sb[:], io["bq"][:])
                nc.sync.dma_start(bk_sb[:], io["bk"][:])
                nc.sync.dma_start(
                    bvbc[:], io["bv"][None, :].to_broadcast((P, NHL * HS))
                )
                nc.sync.dma_start(mask2[:], io["mask2"][:])

                for J in range(NJ):
                    # prefetch ln1 for the next chunk
                    if J < NJ - 1:
                        ln1_pair(2 * J + 2)
                        ln1_pair(2 * J + 3)
                    elif J == NJ - 1:
                        # proj-phase inputs; slack until phase D
                        nc.sync.dma_start(wo_sb[:], io["wo"][:])
                        nc.sync.dma_start(xpb_sb[:], io["xpb"][:])
                        nc.sync.dma_start(b1p_sb[:], io["b1p"][:])
                        nc.sync.dma_start(
                            b2bc_sb[:], io["b2"][None, :].to_broadcast((P, C))
                        )
                    for m in range(4):
                        mt = 4 * J + m
                        vps = qkpsum.tile([P, 512], F32, tag="qkv_ps", name="vps")
                        for kt in range(KT):
                            nc.tensor.matmul(
                                vps[:, :NHL * HS],
                                hT_sb[:, kt, mt * P:(mt + 1) * P],
                                wv_sb[:, kt, :],
                                start=(kt == 0), stop=(kt == KT - 1),
                            )
                        nc.vector.tensor_add(
                            v_sb[:, mt, :, 0:HS],
                            vps[:, :NHL * HS].rearrange(
                                "p (h d) -> p h d", h=NHL
                            ),
                            bvbc[:].rearrange("p (h d) -> p h d", h=NHL),
                        )
                    for pair in range(NPAIR):
                        qps = qkpsum.tile([P, 512], F32, tag="qkv_ps", name="qps")
                        for kt in range(KT):
                            nc.tensor.matmul(
                                qps[:], wq_sb[:, kt, pair, :],
                                hT_sb[:, kt, J * 512:(J + 1) * 512],
                                start=(kt == 0), stop=(kt == KT - 1),
                            )
                        nc.scalar.activation(
                            out=qT_sb[:, pair, J * 512:(J + 1) * 512], in_=qps[:],
                            func=ID, bias=bq_sb[:, pair:pair + 1],
                        )
                        kps = qkpsum.tile([P, 512], F32, tag="qkv_ps", name="kps")
                        for kt in range(KT):
                            nc.tensor.matmul(
                                kps[:], wk_sb[:, kt, pair, :],
                                hT_sb[:, kt, J * 512:(J + 1) * 512],
                                start=(kt == 0), stop=(kt == KT - 1),
                            )
                        nc.scalar.activation(
                            out=kT_sb[:, pair, J * 512:(J + 1) * 512], in_=kps[:],
                            func=ID, bias=bk_sb[:, pair:pair + 1],
                        )

            # W1 full prefetch (bf16, 8.4MB); DMAs issued here so x/qkv
            # weight loads win the queues first; overlaps attention.
            w1_sb = w1pool.tile([P, MFF, KT * P], BF16)
            w1v = io["w1"].rearrange("m p f -> p m f")
            for m0 in range(0, MFF, 8):
                nc.sync.dma_start(w1_sb[:, m0:m0 + 8, :], w1v[:, m0:m0 + 8, :])

            # ----- Phase C: causal attention (ACT table: Exp) -----
            with ExitStack() as ph:
                stpsum = ph.enter_context(
                    tc.tile_pool(name="stpsum", bufs=2, space="PSUM")
                )
                upsum = ph.enter_context(
                    tc.tile_pool(name="upsum", bufs=2, space="PSUM")
                )
                ppool = ph.enter_context(tc.tile_pool(name="pT", bufs=4))
                npool = ph.enter_context(tc.tile_pool(name="norm", bufs=2))
                for J in range(NJ):
                    for pair in range(NPAIR):
                        nk = 4 * J + 4
                        ups = upsum.tile([P, 2, 512], F32, tag="ups", name="ups")
                        for i in range(nk):
                            d = max(0, i - 4 * J)
                            w = 512 - d * P
                            q0 = J * 512 + d * P
                            sps = stpsum.tile([P, 2, 512], F32, tag="sps",
                                              name="sps")
                            for j in range(2):
                                nc.tensor.matmul(
                                    sps[:, j, :w],
                                    kT_sb[64 * j:64 * j + 64, pair,
                                          i * P:(i + 1) * P],
                                    qT_sb[64 * j:64 * j + 64, pair,
                                          q0:J * 512 + 512],
                                    start=True, stop=True,
                                    tile_position=(64 * j, 0),
                                )
                            if i >= 4 * J:
                                # triangular mask on the diagonal 128-block
                                nc.vector.tensor_add(
                                    sps[:, :, 0:P], sps[:, :, 0:P], mask2[:]
                                )
                            pT = ppool.tile([P, 2, 512], BF16, tag="pT",
                                            name="pT")
                            nc.scalar.activation(
                                out=pT[:, :, :w], in_=sps[:, :, :w], func=EXP
                            )
                            for j in range(2):
                                h = 2 * pair + j
                                nc.tensor.matmul(
                                    ups[0:HS + 1, j, d * P:512],
                                    v_sb[:, i, h, :],
                                    pT[:, j, :w],
                                    start=(i == 0), stop=(i == nk - 1),
                                )
                        att = npool.tile([HS, 2, 512], BF16, tag="att",
                                         name="att")
                        for j in range(2):
                            recip = npool.tile([1, 512], F32, tag="recip",
                                               name="recip")
                            nc.vector.reciprocal(recip[:], ups[HS:HS + 1, j, :])
                            rbc = npool.tile([HS, 512], F32, tag="rbc",
                                             name="rbc")
                            nc.gpsimd.partition_broadcast(rbc[:], recip[:])
                            nc.vector.tensor_mul(
                                att[:, j, :], ups[0:HS, j, :], rbc[:]
                            )
                        h0 = 2 * pair
                        nc.sync.dma_start(
                            ag_in[J, h0 * HS:(h0 + 2) * HS, :].rearrange(
                                "(j p) t -> p j t", p=HS
                            ),
                            att[:],
                        )
                    if fake_collective:
                        for rr in range(GSZ):
                            nc.sync.dma_start(
                                ag_out[J, rr * NHL * HS:(rr + 1) * NHL * HS, :],
                                ag_in[J],
                            )
                    else:
                        nc.gpsimd.collective_compute(
                            "AllGather", mybir.AluOpType.bypass,
                            replica_groups=REPLICA_GROUPS,
                            ins=[ag_in[J].opt()], outs=[ag_out[J].opt()],
                        )

        # ---------- Phases D..G (attention pools freed) ----------
        with ExitStack() as tail:
            tailp = tail.enter_context(tc.tile_pool(name="tailp", bufs=1))
            yhp = tail.enter_context(tc.tile_pool(name="yh", bufs=2))
            yT_sb = tailp.tile([P, KT, TLOC], BF16)
            rT = tailp.tile([P, MFF, TLOC], BF16)

            # ----- Phase D: dynamic gather + proj + residual + inline LN2 ---
            with ExitStack() as ph:
                agp = ph.enter_context(tc.tile_pool(name="agp", bufs=1))
                prpsum = ph.enter_context(
                    tc.tile_pool(name="prpsum", bufs=6, space="PSUM")
                )
                pid = nc.sync.partition_id()
                rsel = pid % GSZ
                ag_sb = agp.tile([P, KT, TLOC], BF16)
                ag_view = ag_out[:].rearrange("j (kt p) t -> p j kt t", p=P)
                nc.sync.dma_start(
                    ag_sb[:], ag_view[:, bass.ds(rsel, 1), :, :]
                )
                for mt in range(MTL):
                    for nt in range(2):
                        pps = prpsum.tile([P, 512], F32, tag="pps", name="pps")
                        for kt in range(KT):
                            nc.tensor.matmul(
                                pps[:], ag_sb[:, kt, mt * P:(mt + 1) * P],
                                wo_sb[:, kt, nt * 512:(nt + 1) * 512],
                                start=(kt == 0), stop=(kt == KT - 1),
                            )
                        nc.vector.tensor_add(
                            y_sb[:, mt, nt * 512:(nt + 1) * 512], pps[:],
                            xpb_sb[:, mt, nt * 512:(nt + 1) * 512],
                        )
                    # inline ln2 + DMA-xbar transpose per token tile, so
                    # FFN1 isn't gated on a serial LN2 pass
                    rstd, nmu = _ln_stats(nc, stat_pool, y_sb[:, mt, :], eps_t)
                    yh = yhp.tile([P, C], BF16, tag="yh", name="yh")
                    nc.vector.tensor_scalar(
                        out=yh[:], in0=y_sb[:, mt, :], scalar1=nmu[:],
                        scalar2=rstd[:],
                        op0=mybir.AluOpType.add, op1=mybir.AluOpType.mult,
                    )
                    nc.scalar.dma_start_transpose(
                        yT_sb[:, :, mt * P:(mt + 1) * P], yh[:]
                    )
                # y := y + b2 in place (residual base for the final add);
                # after the mt loop so it doesn't gate the yT transposes
                for mt in range(MTL):
                    nc.vector.tensor_add(
                        y_sb[:, mt, :], y_sb[:, mt, :], b2bc_sb[:]
                    )

            # ----- Phase F: FFN1 (relu(yT @ W1 + b1) -> rT; ACT table Relu) --
            with ExitStack() as ph:
                zps_p = ph.enter_context(
                    tc.tile_pool(name="zps", bufs=4, space="PSUM")
                )
                for mf in range(MFF):
                    zps = zps_p.tile([P, 512], F32, tag="zps", name="zps")
                    for kt in range(KT):
                        nc.tensor.matmul(
                            zps[:], w1_sb[:, mf, kt * P:(kt + 1) * P],
                            yT_sb[:, kt, :],
                            start=(kt == 0), stop=(kt == KT - 1),
                        )
                    nc.scalar.activation(
                        out=rT[:, mf, :], in_=zps[:], func=RELU,
                        bias=b1p_sb[:, mf:mf + 1],
                    )

            # ----- Phase G: FFN2 + residual + out -----
            w2p = tail.enter_context(tc.tile_pool(name="w2p", bufs=6))
            with ExitStack() as ph:
                fps_p = ph.enter_context(
                    tc.tile_pool(name="fps", bufs=1, space="PSUM")
                )
                otmp = ph.enter_context(tc.tile_pool(name="otmp", bufs=3))
                fps = [
                    [
                        fps_p.tile(
                            [P, 512], F32, tag=f"fps_{mt}_{nt}",
                            name=f"fps_{mt}_{nt}",
                        )
                        for nt in range(2)
                    ]
                    for mt in range(MTL)
                ]
                for kf in range(KF):
                    w2_t = w2p.tile([P, C], BF16, tag="w2t", name="w2t")
                    nc.sync.dma_start(w2_t[:], io["w2"][kf * P:(kf + 1) * P, :])
                    for mt in range(MTL):
                        for nt in range(2):
                            nc.tensor.matmul(
                                fps[mt][nt][:],
                                rT[:, kf, mt * P:(mt + 1) * P],
                                w2_t[:, nt * 512:(nt + 1) * 512],
                                start=(kf == 0), stop=(kf == KF - 1),
                            )
                for mt in range(MTL):
                    t1 = otmp.tile([P, C], BF16, tag="otmp", name="otmp")
                    for nt in range(2):
                        nc.vector.tensor_add(
                            t1[:, nt * 512:(nt + 1) * 512], fps[mt][nt][:],
                            y_sb[:, mt, nt * 512:(nt + 1) * 512],
                        )
                    nc.sync.dma_start(
                        io["out"][mt * P:(mt + 1) * P, :], t1[:]
                    )


def build_nc(niter=1, fake_collective=False):
    nc = bacc.Bacc(None, target_bir_lowering=False, debug=False,
                   num_devices=NCORES)
    io = {}
    io["x"] = nc.dram_tensor("x", [T, C], BF16, kind="ExternalInput").ap()
    io["xpb"] = nc.dram_tensor(
        "xpb", [P, MTL, C], BF16, kind="ExternalInput"
    ).ap()
    io["b2"] = nc.dram_tensor("b2", [C], F32, kind="ExternalInput").ap()
    io["wq"] = nc.dram_tensor(
        "wq", [P, KT, NPAIR, P], BF16, kind="ExternalInput"
    ).ap()
    io["wk"] = nc.dram_tensor(
        "wk", [P, KT, NPAIR, P], BF16, kind="ExternalInput"
    ).ap()
    io["wv"] = nc.dram_tensor(
        "wv", [P, KT, NHL * HS], BF16, kind="ExternalInput"
    ).ap()
    io["bq"] = nc.dram_tensor("bq", [P, NPAIR], F32, kind="ExternalInput").ap()
    io["bk"] = nc.dram_tensor("bk", [P, NPAIR], F32, kind="ExternalInput").ap()
    io["bv"] = nc.dram_tensor("bv", [NHL * HS], F32, kind="ExternalInput").ap()
    io["wo"] = nc.dram_tensor("wo", [P, KT, C], BF16, kind="ExternalInput").ap()
    io["w1"] = nc.dram_tensor(
        "w1", [MFF, P, KT * P], BF16, kind="ExternalInput"
    ).ap()
    io["b1p"] = nc.dram_tensor("b1p", [P, MFF], F32, kind="ExternalInput").ap()
    io["w2"] = nc.dram_tensor("w2", [FF, C], BF16, kind="ExternalInput").ap()
    io["mask2"] = nc.dram_tensor(
        "mask2", [P, 2, P], F32, kind="ExternalInput"
    ).ap()
    io["out"] = nc.dram_tensor("out", [TLOC, C], BF16,
                           kind="ExternalOutput").ap()
    with tile.TileContext(nc) as tc:
        _emit(tc, io, niter, fake_collective)
    nc.compile()
    return nc


def host_prep(inputs):
    """Fold layernorm affines / biases / attention scale into the weights,
    cast to bf16, and build the 8 per-core input maps."""
    f = np.float32
    x = np.ascontiguousarray(inputs["x"], f)
    Wq, Wk, Wv = (np.asarray(inputs[k], f) for k in ("Wq", "Wk", "Wv"))
    Wo, bo = np.asarray(inputs["Wo"], f), np.asarray(inputs["bo"], f)
    W1, b1 = np.asarray(inputs["W1"], f), np.asarray(inputs["b1"], f)
    W2, b2 = np.asarray(inputs["W2"], f), np.asarray(inputs["b2"], f)
    g1, be1 = np.asarray(inputs["g1"], f), np.asarray(inputs["be1"], f)
    g2, be2 = np.asarray(inputs["g2"], f), np.asarray(inputs["be2"], f)

    scale = f(C) ** f(-0.5)
    Wq_f = (g1[None, :, None] * Wq) * scale
    Wk_f = g1[None, :, None] * Wk
    Wv_f = g1[None, :, None] * Wv
    bq = np.einsum("c,hcd->hd", be1, Wq).astype(f) * scale
    bk = np.einsum("c,hcd->hd", be1, Wk).astype(f)
    bv = np.einsum("c,hcd->hd", be1, Wv).astype(f)
    W1_f = np.ascontiguousarray(g2[:, None] * W1, f)
    b1p = (b1 + be2 @ W1).astype(f)
    Wo_c = np.ascontiguousarray(
        Wo.reshape(KT, P, C).transpose(1, 0, 2)
    ).astype(BF16NP)
    # W1 pre-tiled: [mf, p(c within kt), kt*128(ff within mf)]
    W1_t = np.ascontiguousarray(
        W1_f.reshape(KT, P, MFF, P).transpose(2, 1, 0, 3).reshape(MFF, P, KT * P)
    ).astype(BF16NP)
    W2_c = np.ascontiguousarray(W2).astype(BF16NP)
    b1p_dev = np.ascontiguousarray(b1p.reshape(MFF, P).T)

    # triangular mask for the diagonal 128-block: keep q >= k
    rr = np.arange(P)[:, None]
    cc = np.arange(P)[None, :]
    tri = np.where(cc - rr >= 0, 0.0, -1e9).astype(f)
    mask2_np = np.ascontiguousarray(np.stack([tri, tri], axis=1))

    in_maps = []
    for c in range(NCORES):
        g, r = divmod(c, GSZ)
        hs = [GSZ * r + j for j in range(NHL)]
        wq_pairs = np.stack(
            [np.concatenate([Wq_f[hs[2 * p]], Wq_f[hs[2 * p + 1]]], axis=1)
             for p in range(NPAIR)]
        )
        wk_pairs = np.stack(
            [np.concatenate([Wk_f[hs[2 * p]], Wk_f[hs[2 * p + 1]]], axis=1)
             for p in range(NPAIR)]
        )
        bq_pairs = np.stack(
            [np.concatenate([bq[hs[2 * p]], bq[hs[2 * p + 1]]])
             for p in range(NPAIR)]
        )
        bk_pairs = np.stack(
            [np.concatenate([bk[hs[2 * p]], bk[hs[2 * p + 1]]])
             for p in range(NPAIR)]
        )
        wv_cat = np.concatenate([Wv_f[h] for h in hs], axis=1)
        x_loc = x[g, TLOC * r:TLOC * (r + 1)]
        xpb = x_loc + bo
        in_maps.append({
            "x": np.ascontiguousarray(x[g]).astype(BF16NP),
            "xpb": np.ascontiguousarray(
                xpb.reshape(MTL, P, C).transpose(1, 0, 2)
            ).astype(BF16NP),
            "b2": b2,
            "wq": np.ascontiguousarray(
                wq_pairs.reshape(NPAIR, KT, P, P).transpose(2, 1, 0, 3)
            ).astype(BF16NP),
            "wk": np.ascontiguousarray(
                wk_pairs.reshape(NPAIR, KT, P, P).transpose(2, 1, 0, 3)
            ).astype(BF16NP),
            "wv": np.ascontiguousarray(
                wv_cat.reshape(KT, P, NHL * HS).transpose(1, 0, 2)
            ).astype(BF16NP),
            "bq": np.ascontiguousarray(bq_pairs.T),
            "bk": np.ascontiguousarray(bk_pairs.T),
            "bv": np.ascontiguousarray(np.concatenate([bv[h] for h in hs])),
            "wo": Wo_c,
            "w1": W1_t,
            "b1p": b1p_dev,
            "w2": W2_c,
            "mask2": mask2_np,
        })
    return in_maps


_NC = None


def _get_nc():
    global _NC
    if _NC is None:
        _NC = build_nc()
    return _NC


def kernel(**inputs) -> np.ndarray:
    nc = _get_nc()
    in_maps = host_prep(inputs)
    res = run_bass_kernel_spmd(nc, in_maps, core_ids=list(range(NCORES)))
    out = np.empty((B, T, C), np.float32)
    for c in range(NCORES):
        g, r = divmod(c, GSZ)
        out[g, TLOC * r:TLOC * (r + 1)] = res.results[c]["out"]
    return out
